# revision 12
# baseline (speedup 1.0000x reference)
"""Contextual-attention kernel for Trainium2, 8 NeuronCores, SPMD.

Fully on-device pipeline (transfer-minimal: ~2MB up / ~0.5MB down per core):
  G[tap,c,p]   = clipped 3x3 box-sum of tap-shifted fg            (vector)
  vs3[c,dj,y,x]= padded bg + EPS, x-shifted per dj                (vector)
  U[dj]        = per-row-pair transpose of vs3 (kernel matrix V^T) (PE)
  rn[l]        = 1/sqrt(boxsum(sum_c vs3^2))                      (PE+vector)
  E2[l,p]      = rn * exp(rn * sum_tap vs3_slice^T @ G_tap)       (PE+scalar)
  Z[p]         = sum_l (1/rn)*E2   (matmul with snorm lhsT)       (PE)
  MpT[kk,p]    = (sum_l V[l,kk]*E2[l,p]) / Z[p]                   (PE+vector)
  out          = col2im(MpT) * m/9 + bg                           (vector)

Sharding: core c handles sample c//2, spatial half c%2. The h=1 half is
flipped vertically on host so all cores run the identical program
(reflection equivariance of the operator); host flips outputs back.
"""
import sys
for _p in ('/opt/trn_rl_repo',):
    if _p not in sys.path:
        sys.path.insert(0, _p)

import numpy as np
import ml_dtypes

import concourse.bass as bass
import concourse.mybir as mybir
import concourse.tile as tile
from concourse import bacc
from concourse.bass_utils import run_bass_kernel_spmd

EPS = 1e-7
C, H, W = 128, 64, 64
L = H * W                       # 4096
NROW = 34                       # grid rows of scores/Mz computed per core
NP = NROW * 64                  # 2176 pixels per core (32 out rows + halo + pad)
LT = 32                         # l-tiles of 128 (full L on every core)
CHUNKS = [512, 512, 512, 512, 128]   # p-chunks covering NP
F32 = mybir.dt.float32
BF16 = mybir.dt.bfloat16
MULT = mybir.AluOpType.mult
ADD = mybir.AluOpType.add
EXP = mybir.ActivationFunctionType.Exp
SQUARE = mybir.ActivationFunctionType.Square
SQRT = mybir.ActivationFunctionType.Sqrt

_compiled = None


def _build_program():
    nc = bacc.Bacc("TRN2", target_bir_lowering=False, debug=False)
    fg_d = nc.dram_tensor("fg", [C, H, W], BF16, kind="ExternalInput").ap()
    om_d = nc.dram_tensor("om", [1, L], F32, kind="ExternalInput").ap()
    m9_d = nc.dram_tensor("m9", [1, 2048], F32, kind="ExternalInput").ap()
    ident_d = nc.dram_tensor("ident", [C, C], BF16, kind="ExternalInput").ap()
    out_d = nc.dram_tensor("out", [C, 32, W], BF16, kind="ExternalOutput").ap()

    with tile.TileContext(nc) as tc:
        with (
            tc.tile_pool(name="per", bufs=1) as per,
            tc.tile_pool(name="scr", bufs=4) as scr,
            tc.tile_pool(name="ps1p", bufs=2, space="PSUM") as ps1p,
            tc.tile_pool(name="ps2p", bufs=2, space="PSUM") as ps2p,
            tc.tile_pool(name="psSp", bufs=1, space="PSUM") as psSp,
            tc.tile_pool(name="psTp", bufs=2, space="PSUM") as psTp,
        ):
            # ---------- constants ----------
            ident = per.tile([C, C], BF16, tag="ident")
            nc.sync.dma_start(out=ident[:], in_=ident_d[:])
            onesmat = per.tile([C, C], BF16, tag="onesmat")
            nc.vector.memset(onesmat[:], 1.0)
            ones1 = per.tile([1, C], F32, tag="ones1")
            nc.vector.memset(ones1[:], 1.0)

            # ---------- load fg ----------
            fgt = per.tile([C, H, W], BF16, tag="fgt")
            nc.sync.dma_start(out=fgt[:], in_=fg_d[:])

            # ---------- G build: R(dj) then 9 taps ----------
            G = per.tile([C, 9, NROW, W], BF16, tag="G")
            nc.vector.memset(G[:], 0.0)
            for dj in range(3):
                R = scr.tile([C, H, W], BF16, tag="scr", name=f"R{dj}")
                nc.vector.memset(R[:], 0.0)
                for t in (-1, 0, 1):
                    xs = max(0, 1 - t - dj)
                    xe = min(64, 65 - t - dj)
                    if xs < xe:
                        nc.vector.tensor_tensor(
                            out=R[:, :, xs:xe], in0=R[:, :, xs:xe],
                            in1=fgt[:, :, xs + t + dj - 1:xe + t + dj - 1], op=ADD)
                for di in range(3):
                    tap = di * 3 + dj
                    for t in (-1, 0, 1):
                        ga = max(0, -t, 1 - di - t)
                        gb = min(63, 63 - t, 64 - di - t)
                        yys, yye = ga, min(NROW, gb + 1)
                        if yys < yye:
                            rs = yys + t + di - 1
                            nc.vector.tensor_tensor(
                                out=G[:, tap, yys:yye, :], in0=G[:, tap, yys:yye, :],
                                in1=R[:, rs:rs + (yye - yys), :], op=ADD)

            # ---------- bg = fg * (1-m), in place ----------
            for hh in range(2):
                omt = scr.tile([1, 2048], F32, tag="scr", name=f"om{hh}")
                nc.sync.dma_start(out=omt[:], in_=om_d[:, hh * 2048:(hh + 1) * 2048])
                for q in range(4):
                    pb = psSp.tile([C, 8, W], F32, tag="psB", name=f"bgb{hh}_{q}")
                    nc.tensor.matmul(pb[:], ones1[:], omt[:, q * 512:(q + 1) * 512],
                                     start=True, stop=True)
                    r0 = hh * 16 + q * 4
                    nc.vector.tensor_tensor(out=fgt[:, r0 * 2:r0 * 2 + 8, :],
                                            in0=fgt[:, r0 * 2:r0 * 2 + 8, :],
                                            in1=pb[:], op=MULT)

            # ---------- vs3 = padded bg + EPS (x pre-shifted per dj) ----------
            vs3 = per.tile([C, 3, 66, W], BF16, tag="vs3")
            nc.vector.memset(vs3[:], EPS)
            for dj in range(3):
                xs = max(0, 1 - dj)
                xe = min(64, 65 - dj)
                nc.vector.tensor_scalar(
                    out=vs3[:, dj, 1:65, xs:xe],
                    in0=fgt[:, :, xs + dj - 1:xe + dj - 1],
                    scalar1=EPS, scalar2=None, op0=ADD)

            # ---------- U[dj][(r,x), q, c] = vs3[c, dj, 2q+r, x] ----------
            U = per.tile([C, 3, 33, C], BF16, tag="U")
            Uo = per.tile([C, 3, 32, C], BF16, tag="Uo")
            for dj in range(3):
                with tc.For_i(0, 33, 1) as q:
                    pt = psTp.tile([C, C], BF16, tag="psT", name=f"ut{dj}")
                    nc.tensor.transpose(pt[:], vs3[:, dj, ds(2 * q, 2), :], ident[:])
                    nc.vector.tensor_copy(out=U[:, dj, ds(q, 1), :], in_=pt[:])
                with tc.For_i(0, 32, 1) as q:
                    pt = psTp.tile([C, C], BF16, tag="psT", name=f"uo{dj}")
                    nc.tensor.transpose(pt[:], vs3[:, dj, ds(2 * q + 1, 2), :],
                                        ident[:])
                    nc.vector.tensor_copy(out=Uo[:, dj, ds(q, 1), :], in_=pt[:])

            # ---------- rn: norm of kernels via boxsum of channel-summed squares ----
            sq = scr.tile([C, 66, 66], BF16, tag="scr", name="sq")
            nc.scalar.activation(sq[:, :, 0:64], vs3[:, 0, :, :], SQUARE)
            nc.scalar.activation(sq[:, :, 64:65], vs3[:, 1, :, 63:64], SQUARE)
            nc.scalar.activation(sq[:, :, 65:66], vs3[:, 2, :, 63:64], SQUARE)
            s2 = scr.tile([C, 66, 66], BF16, tag="scr", name="s2")
            for qc in range(10):
                r0, r1 = qc * 7, min(66, qc * 7 + 7)
                pb = psSp.tile([C, 512], F32, tag="psB", name=f"cs{qc}")
                nc.tensor.matmul(pb[:, :(r1 - r0) * 66], onesmat[:], sq[:, r0:r1, :],
                                 start=True, stop=True)
                nc.vector.tensor_copy(out=s2[:, r0:r1, :], in_=pb[:, :(r1 - r0) * 66])
            t1 = scr.tile([C, 64, 66], BF16, tag="scr", name="t1")
            nc.vector.tensor_copy(out=t1[:], in_=s2[:, 0:64, :])
            nc.vector.tensor_tensor(out=t1[:], in0=t1[:], in1=s2[:, 1:65, :], op=ADD)
            nc.vector.tensor_tensor(out=t1[:], in0=t1[:], in1=s2[:, 2:66, :], op=ADD)
            ns = scr.tile([C, 64, 64], BF16, tag="scr", name="ns")
            nc.vector.tensor_copy(out=ns[:], in_=t1[:, :, 0:64])
            nc.vector.tensor_tensor(out=ns[:], in0=ns[:], in1=t1[:, :, 1:65], op=ADD)
            nc.vector.tensor_tensor(out=ns[:], in0=ns[:], in1=t1[:, :, 2:66], op=ADD)
            nsq = per.tile([C, LT], F32, tag="nsq")
            for lt in range(LT):
                pt = psTp.tile([C, C], BF16, tag="psT", name=f"nt{lt}")
                nc.tensor.transpose(pt[:], ns[:, 2 * lt:2 * lt + 2, :], ident[:])
                nc.vector.tensor_copy(out=nsq[:, lt:lt + 1], in_=pt[:, 0:1])
            snorm = per.tile([C, LT], F32, tag="snorm")
            nc.scalar.activation(snorm[:], nsq[:], SQRT)
            snb = per.tile([C, LT], BF16, tag="snb")
            nc.vector.tensor_copy(out=snb[:], in_=snorm[:])
            rnt = per.tile([C, LT], F32, tag="rnt")
            nc.vector.reciprocal(rnt[:], snorm[:])

            # ---------- main chunk loop ----------
            ss = per.tile([C, LT, 512], BF16, tag="ss")
            rec = per.tile([C, 32, W], F32, tag="rec")
            nc.vector.memset(rec[:], 0.0)
            pc0 = 0
            for ci, cw in enumerate(CHUNKS):
                r0 = pc0 // 64
                nr = cw // 64
                # matmul1 + exp(rn*s) + rn scale
                for lt in range(LT):
                    p1 = ps1p.tile([C, 512], F32, tag="ps1", name=f"p1_{ci}_{lt}")
                    for tap in range(9):
                        di, dj = tap // 3, tap % 3
                        nc.tensor.matmul(p1[:, :cw],
                                         vs3[:, dj, 2 * lt + di:2 * lt + di + 2, :],
                                         G[:, tap, r0:r0 + nr, :],
                                         start=(tap == 0), stop=(tap == 8))
                    nc.scalar.activation(ss[:, lt, :cw], p1[:, :cw], EXP,
                                         scale=rnt[:, lt:lt + 1])
                    nc.vector.tensor_scalar(out=ss[:, lt, :cw], in0=ss[:, lt, :cw],
                                            scalar1=rnt[:, lt:lt + 1], scalar2=None,
                                            op0=MULT)
                # Z = sum_l E  (E = ss/rn, via snorm lhsT)
                pz = psSp.tile([1, 512], F32, tag="psZ", name=f"pz{ci}")
                for lt in range(LT):
                    nc.tensor.matmul(pz[:, :cw], snb[:, lt:lt + 1], ss[:, lt, :cw],
                                     start=(lt == 0), stop=(lt == LT - 1))
                rz = scr.tile([1, 512], F32, tag="scr", name=f"rz{ci}")
                nc.vector.reciprocal(rz[:, :cw], pz[:, :cw])
                pb = psSp.tile([C, 8, W], F32, tag="psB", name=f"zb{ci}")
                nc.tensor.matmul(pb[:, :nr, :], ones1[:], rz[:, :cw],
                                 start=True, stop=True)
                rzb = scr.tile([C, 8, W], F32, tag="scr", name=f"rzb{ci}")
                nc.vector.tensor_copy(out=rzb[:, :nr, :], in_=pb[:, :nr, :])
                # MzT per tap -> divide by Z -> col2im into rec
                for tap in range(9):
                    di, dj = tap // 3, tap % 3
                    p2 = ps2p.tile([C, 8, W], F32, tag="ps2", name=f"p2_{ci}_{tap}")
                    if di in (0, 2):
                        for lt in range(LT):
                            nc.tensor.matmul(p2[:, :nr, :],
                                             U[:, dj, lt + di // 2, :],
                                             ss[:, lt, :cw],
                                             start=(lt == 0), stop=(lt == LT - 1))
                    else:
                        for lt in range(LT):
                            nc.tensor.matmul(p2[:, :nr, :],
                                             Uo[:, dj, lt, :], ss[:, lt, :cw],
                                             start=(lt == 0), stop=(lt == LT - 1))
                    mp = scr.tile([C, 8, W], F32, tag="scr", name=f"mp{ci}_{tap}")
                    nc.vector.tensor_tensor(out=mp[:, :nr, :], in0=p2[:, :nr, :],
                                            in1=rzb[:, :nr, :], op=MULT)
                    # col2im: out row y = r0+rr+di-1 in [0,32)
                    rr0 = max(0, 1 - di - r0)
                    rr1 = min(nr, 33 - di - r0)
                    if dj == 0:
                        pxs, pxe, xd = 1, 64, 0
                    elif dj == 1:
                        pxs, pxe, xd = 0, 64, 0
                    else:
                        pxs, pxe, xd = 0, 63, 1
                    if rr0 < rr1:
                        ya = r0 + rr0 + di - 1
                        yb = r0 + rr1 + di - 1
                        nc.vector.tensor_tensor(
                            out=rec[:, ya:yb, xd:xd + (pxe - pxs)],
                            in0=rec[:, ya:yb, xd:xd + (pxe - pxs)],
                            in1=mp[:, rr0:rr1, pxs:pxe], op=ADD)
                pc0 += cw

            # ---------- final: out = rec * m9 + bg ----------
            m9t = scr.tile([1, 2048], F32, tag="scr", name="m9t")
            nc.sync.dma_start(out=m9t[:], in_=m9_d[:])
            outt = scr.tile([C, 32, W], BF16, tag="scr", name="outt")
            for q in range(4):
                pb = psSp.tile([C, 8, W], F32, tag="psB", name=f"m9b{q}")
                nc.tensor.matmul(pb[:], ones1[:], m9t[:, q * 512:(q + 1) * 512],
                                 start=True, stop=True)
                tmp = scr.tile([C, 8, W], F32, tag="scr", name=f"fin{q}")
                nc.vector.tensor_tensor(out=tmp[:], in0=rec[:, q * 8:q * 8 + 8, :],
                                        in1=pb[:], op=MULT)
                nc.vector.tensor_tensor(out=outt[:, q * 8:q * 8 + 8, :],
                                        in0=tmp[:], in1=fgt[:, q * 8:q * 8 + 8, :],
                                        op=ADD)
            nc.sync.dma_start(out=out_d[:], in_=outt[:])
    nc.compile()
    return nc


def kernel(foreground, mask, _results_hook=None):
    global _compiled
    fg = np.asarray(foreground, np.float32)
    m = np.asarray(mask, np.float32)
    B = fg.shape[0]

    if _compiled is None:
        _compiled = _build_program()
    nc = _compiled

    ident = np.eye(C, dtype=ml_dtypes.bfloat16)
    in_maps = []
    for core in range(2 * B):
        s, h = core // 2, core % 2
        f, mm = fg[s], m[s]
        if h == 1:
            f = f[:, ::-1, :]
            mm = mm[:, ::-1, :]
        in_maps.append({
            "fg": np.ascontiguousarray(f.astype(ml_dtypes.bfloat16)),
            "om": (1.0 - mm).reshape(1, L).astype(np.float32),
            "m9": np.ascontiguousarray((mm / 9.0).reshape(1, L)[:, :2048]).astype(np.float32),
            "ident": ident,
        })

    res = run_bass_kernel_spmd(nc, in_maps, list(range(2 * B)))
    if _results_hook is not None:
        _results_hook(res)

    out = np.empty((B, C, H, W), np.float32)
    for core in range(2 * B):
        s, h = core // 2, core % 2
        o = np.asarray(res.results[core]["out"]).astype(np.float32)
        if h == 0:
            out[s, :, 0:32, :] = o
        else:
            out[s, :, 32:64, :] = o[:, ::-1, :]
    return out
